# revision 25
# baseline (speedup 1.0000x reference)
"""LoFTR LocalFeatureTransformer as a hand-written Bass/Tile SPMD kernel.

8 NeuronCores, one sequence per core (core i: feat{i%2}[i//2]); cross
layers exchange linear-attention KV statistics ([128,2,129] f32) with a
pairwise AllReduce (partner = sum - own).

The residual stream lives TRANSPOSED in SBUF in an interleaved layout
xT [128, NCH, 2, 128] bf16: element (p, c, t, l) = x[128*c + l,
128*t + p] — i.e. channel-within-half on partitions, then (l-chunk,
channel-half, l-within-chunk) along the free axis. This layout is what
one full-tensor DMA-xbar transpose of the natural [128, NCH, 256]
chunked layout produces, so natural<->transposed conversions are a
single hidden DMA op per tensor per layer instead of 38 PE transposes +
copies. LayerNorm runs in natural layout.

v2 changes (engine balance / instruction-count reduction):
- elu(x)+1 computed as max(x,0) + min(exp(x),1): exp on ACT straight
  from PSUM, relu on DVE, min+add as one scalar_tensor_tensor on
  GPSIMD (otherwise idle). No serial RELU->EXP chain on one engine.
- K/V projection runs on 2-chunk PSUM pairs ([128,2,512], 2 banks) so
  every elementwise op covers 512 free elements - the ~150-250 ns
  fixed cost per ACT/DVE instruction is paid half as often.
- Q projection per span in one [128,2,512] PSUM tile; elu ops batched
  over both output halves ([128,1024] free).
- attention Z: single tensor_tensor divide (msg/den) instead of
  reciprocal_approx + multiply.
- LN stats via 2-chunk-batched bn_stats ([128,2,256] -> [128,2,6]);
  ln/exp/nmr small-ops batched per 4-chunk group; LN applies rotated
  ACT:DVE 3:1.
- stats matmuls interleaved into the K/V pair loop so the PE queue
  stays dense (HAM stays at K=8/8).
- dummy AllReduce warm-up during layer 0 to pay collective setup cost
  off the critical path.

All matmuls bf16 / fp32 PSUM. LN1 gamma/beta folded exactly into W1's
msg half + bias; LN2 general path only when gamma/beta not ones/zeros.
The activation-table pass is pinned to natural_log_exp_and_others
(covers exp/ln/relu/copy/identity) — the stock greedy pass thrashes
1.28us table reloads between exp and ln tables.
"""

from contextlib import ExitStack

import numpy as np
import ml_dtypes

import concourse.bass as bass
import concourse.mybir as mybir
import concourse.tile as tile
from concourse import bacc
from concourse.hw_specs import get_activation_tables

F32 = mybir.dt.float32
BF16 = mybir.dt.bfloat16
AF = mybir.ActivationFunctionType
ALU = mybir.AluOpType

D_MODEL = 256
LN_EPS = 1e-5
N_CORES = 8


class _BaccOneTable(bacc.Bacc):
    ACT_TABLE = "natural_log_exp_and_others"

    def insert_act_table_loads(self):
        has_activation = any(
            isinstance(i, mybir.InstActivation)
            for b in self.main_func.blocks
            for i in b.instructions)
        if not has_activation:
            return
        tables = [(n, (s if n == self.ACT_TABLE else set()))
                  for n, s in get_activation_tables(self.m.arch).items()]
        bacc._bass_rust.insert_act_table_loads(self, tables)


def _spans(LP, step=512):
    return [(s, min(step, LP - s)) for s in range(0, LP, step)]


def prep_weights(Wq, Wk, Wv, Wm, W1, W2, g1, b1, g2, b2):
    bf = ml_dtypes.bfloat16

    def tile_w(w):
        nl, fi, fo = w.shape
        return np.ascontiguousarray(
            w.reshape(nl, fi // 128, 128, fo)).astype(bf)

    W1f = np.array(W1, dtype=np.float32, copy=True)
    W1f[:, D_MODEL:, :] *= g1[:, :, None]
    bias1 = np.einsum("lc,lco->lo", b1, W1[:, D_MODEL:, :]).astype(np.float32)
    g2b2_general = not (np.allclose(g2, 1.0) and np.allclose(b2, 0.0))

    mask = np.zeros((128, 2, 129), dtype=bf)
    for b in range(4):
        mask[32 * b:32 * (b + 1), :, 32 * b:32 * (b + 1)] = 1.0
    mask[:, :, 128] = 1.0

    Wkv = np.concatenate([Wk, Wv], axis=2)  # [NL, 256, 512]
    wmap = {
        "wq": tile_w(Wq), "wkv": tile_w(Wkv), "wm": tile_w(Wm),
        "w1": tile_w(W1f), "w2": tile_w(W2),
        "bias1": bias1, "maskc": mask,
    }
    if g2b2_general:
        wmap["g2"] = g2.astype(np.float32)
        wmap["b2"] = b2.astype(np.float32)
    return wmap, g2b2_general


def build(L, kinds, g2b2_general):
    NL = len(kinds)
    LP = -(-L // 128) * 128
    NCH = LP // 128
    last_rows = L - 128 * (NCH - 1)
    spans = _spans(LP)

    nc = _BaccOneTable()
    x_in = nc.declare_dram_parameter("x", [L, D_MODEL], F32, isOutput=False)
    wq_in = nc.declare_dram_parameter("wq", [NL, 2, 128, 256], BF16, isOutput=False)
    wkv_in = nc.declare_dram_parameter("wkv", [NL, 2, 128, 512], BF16, isOutput=False)
    wm_in = nc.declare_dram_parameter("wm", [NL, 2, 128, 256], BF16, isOutput=False)
    w1_in = nc.declare_dram_parameter("w1", [NL, 4, 128, 512], BF16, isOutput=False)
    w2_in = nc.declare_dram_parameter("w2", [NL, 4, 128, 256], BF16, isOutput=False)
    b1_in = nc.declare_dram_parameter("bias1", [NL, 512], F32, isOutput=False)
    mk_in = nc.declare_dram_parameter("maskc", [128, 2, 129], BF16, isOutput=False)
    if g2b2_general:
        g2_in = nc.declare_dram_parameter("g2", [NL, 256], F32, isOutput=False)
        b2_in = nc.declare_dram_parameter("b2", [NL, 256], F32, isOutput=False)
    out_d = nc.declare_dram_parameter("out", [L, D_MODEL], F32, isOutput=True)

    n_cross = sum(1 for k in kinds if k == "cross")
    cc_in = [nc.dram_tensor(f"cc_in{i}", [128, 258], F32) for i in range(n_cross)]
    cc_out = [nc.dram_tensor(f"cc_out{i}", [128, 258], F32) for i in range(n_cross)]
    ccw_in = nc.dram_tensor("ccw_in", [128, 8], F32)
    ccw_out = nc.dram_tensor("ccw_out", [128, 8], F32)
    groups = [[2 * i, 2 * i + 1] for i in range(N_CORES // 2)]

    with ExitStack() as ctx:
        tc = ctx.enter_context(tile.TileContext(nc))
        cons = ctx.enter_context(tc.tile_pool(name="cons", bufs=1))
        wpool = ctx.enter_context(tc.tile_pool(name="wts", bufs=2))
        xtp = ctx.enter_context(tc.tile_pool(name="xtp", bufs=2))
        qp = ctx.enter_context(tc.tile_pool(name="qp", bufs=1))
        big = ctx.enter_context(tc.tile_pool(name="big", bufs=3))
        h1p = ctx.enter_context(tc.tile_pool(name="h1p", bufs=2))
        sm = ctx.enter_context(tc.tile_pool(name="sm", bufs=2))
        stp = ctx.enter_context(tc.tile_pool(name="stp", bufs=2))
        # PSUM: 8 banks total = ps2 2x[128,1024] (4) + psm 2x[128,512] (2)
        # + psst 1x[128,2,512] (2; one bank per stats half -- a start=True
        # matmul clears the whole bank's has_written bits, so the two
        # interleaved h-accumulations must not share a bank)
        ps2 = ctx.enter_context(tc.tile_pool(name="ps2", bufs=2, space="PSUM"))
        psm = ctx.enter_context(tc.tile_pool(name="psm", bufs=2, space="PSUM"))
        psst = ctx.enter_context(tc.tile_pool(name="psst", bufs=1, space="PSUM"))

        maskc = cons.tile([128, 2, 129], BF16)
        nc.sync.dma_start(out=maskc, in_=mk_in[:, :, :])
        epsc = cons.tile([128, 1], F32)
        nc.vector.memset(epsc, LN_EPS)

        # rhs AP for (t, span) of an interleaved transposed tensor
        def tsl(xt, t, s0, sw):
            return xt[:, s0 // 128:(s0 + sw) // 128, t, :]

        def dma_T(dst, srcn):
            # grouped transpose: src natural [128, NCH, 256] -> dst interleaved
            for g0 in range(0, NCH, 10):
                gn = min(10, NCH - g0)
                nc.sync.dma_start_transpose(
                    out=dst[:, g0:g0 + gn, :, :], in_=srcn[:, g0:g0 + gn, :])

        def dma_Tinv(dstn, src):
            # grouped transpose: src interleaved -> dst natural
            for g0 in range(0, NCH, 10):
                gn = min(10, NCH - g0)
                nc.sync.dma_start_transpose(
                    out=dstn[:, g0:g0 + gn, :, :], in_=src[:, g0:g0 + gn, :, :])

        # ---- load input, cast bf16, one batched transpose -> xT ----
        xnat = big.tile([128, NCH, 256], BF16, tag="big")
        for c in range(0, NCH, 2):
            cn = min(2, NCH - c)
            xin = sm.tile([128, 2, 256], F32, tag="xin")
            for j in range(cn):
                rows = last_rows if c + j == NCH - 1 else 128
                if rows < 128:
                    nc.vector.memset(xin[:, j, :], 0.0)
                nc.sync.dma_start(out=xin[:rows, j, :],
                                  in_=x_in[128 * (c + j):128 * (c + j) + rows, :])
            nc.vector.tensor_copy(out=xnat[:, c:c + cn, :], in_=xin[:, :cn, :])
        xT = xtp.tile([128, NCH, 2, 128], BF16, tag="xT")
        dma_T(xT, xnat)

        # warm up the collective path off the critical path (layer 0 is
        # 'self'; first real AllReduce is layer 1)
        ccw_t = sm.tile([128, 8], F32, tag="ccw")
        nc.vector.memset(ccw_t, 0.0)
        nc.gpsimd.dma_start(out=ccw_in[:, :], in_=ccw_t)
        nc.gpsimd.collective_compute(
            "AllReduce", ALU.add, ins=[ccw_in[:, :]], outs=[ccw_out[:, :]],
            replica_groups=groups)
        ccw_r = sm.tile([128, 8], F32, tag="ccwr")
        nc.gpsimd.dma_start(out=ccw_r, in_=ccw_out[:, :])

        cross_idx = 0
        for li, kind in enumerate(kinds):
            wq = wpool.tile([128, 2, 256], BF16, tag="wq")
            wkv = wpool.tile([128, 2, 512], BF16, tag="wkv")
            wm = wpool.tile([128, 2, 256], BF16, tag="wm")
            w1 = wpool.tile([128, 4, 512], BF16, tag="w1")
            w2 = wpool.tile([128, 4, 256], BF16, tag="w2")
            b1s = wpool.tile([128, 4], F32, tag="b1s")
            for sb_t, dr in ((wq, wq_in), (wkv, wkv_in), (wm, wm_in),
                             (w1, w1_in), (w2, w2_in)):
                nc.sync.dma_start(out=sb_t, in_=dr[li].rearrange("t p n -> p t n"))
            nc.sync.dma_start(out=b1s, in_=b1_in[li].rearrange("(m p) -> p m", p=128))
            if g2b2_general:
                g2r = wpool.tile([128, 256], F32, tag="g2r")
                b2r = wpool.tile([128, 256], F32, tag="b2r")
                nc.sync.dma_start(out=g2r,
                                  in_=g2_in[li:li + 1, :].to_broadcast((128, 256)))
                nc.sync.dma_start(out=b2r,
                                  in_=b2_in[li:li + 1, :].to_broadcast((128, 256)))

            # ---- stage B (first half): q^T + elu -> QT ----
            QT = qp.tile([128, NCH, 2, 128], BF16, tag="QT")

            def emit_B(s0, sw):
                ncl = sw // 128
                qp2 = ps2.tile([128, 2, 512], F32, tag="big2")
                for m in range(2):
                    for t in range(2):
                        nc.tensor.matmul(qp2[:, m, :sw],
                                         wq[:, t, 128 * m:128 * (m + 1)],
                                         tsl(xT, t, s0, sw),
                                         start=(t == 0), stop=(t == 1))
                e2 = sm.tile([128, 2, 512], F32, tag="eQ", bufs=2)
                nc.scalar.activation(out=e2[:, :, :sw], in_=qp2[:, :, :sw],
                                     func=AF.Exp)
                rr = sm.tile([128, 2, 512], F32, tag="rQ", bufs=2)
                nc.scalar.activation(out=rr[:, 0, :sw], in_=qp2[:, 0, :sw],
                                     func=AF.Relu)
                nc.vector.tensor_scalar(out=rr[:, 1, :sw], in0=qp2[:, 1, :sw],
                                        scalar1=0.0, scalar2=None, op0=ALU.max)
                for m in range(2):
                    nc.vector.scalar_tensor_tensor(
                        out=QT[:, s0 // 128:s0 // 128 + ncl, m, :],
                        in0=e2[:, m, :sw], scalar=1.0,
                        in1=rr[:, m, :sw], op0=ALU.min, op1=ALU.add)

            for (s0, sw) in spans[:5]:
                emit_B(s0, sw)

            # ---- stage A: k|v fused matmul on 2-chunk pairs; elu(K);
            # stats interleaved.  elu(x)+1 = min(exp(x),1) + max(x,0) ----
            K = big.tile([128, NCH, 256], BF16, tag="big")
            V = big.tile([128, NCH, 2, 129], BF16, tag="big")
            nc.vector.memset(V[:, :, :, 128:129], 1.0)
            st = psst.tile([128, 2, 512], F32, tag="st")

            def emit_stats(c0, pn):
                for j in range(pn):
                    c = c0 + j
                    for h in range(2):
                        nc.tensor.matmul(st[:, h, 0:129],
                                         K[:, c, 128 * h:128 * (h + 1)],
                                         V[:, c, h, :],
                                         start=(c == 0), stop=(c == NCH - 1))

            pairs = [(c0, min(2, NCH - c0)) for c0 in range(0, NCH, 2)]
            for pi, (c0, pn) in enumerate(pairs):
                kvp = ps2.tile([128, 2, 512], F32, tag="big2")
                for j in range(pn):
                    for t in range(2):
                        nc.tensor.matmul(kvp[:, j, :], xT[:, c0 + j, t, :],
                                         wkv[:, t, :],
                                         start=(t == 0), stop=(t == 1))
                kk = kvp[:, :pn, 0:256]
                # elu(k)+1 = min(exp(k),1) + max(k,0); f32 intermediates.
                # exp and relu both read PSUM directly (slot frees after 2
                # parallel hops), one stt combines from SBUF.
                e2 = sm.tile([128, 2, 256], F32, tag="eK", bufs=2)
                nc.scalar.activation(out=e2[:, :pn, :], in_=kk, func=AF.Exp)
                rr = sm.tile([128, 2, 256], F32, tag="rK", bufs=2)
                nc.vector.tensor_scalar(out=rr[:, :pn, :], in0=kk,
                                        scalar1=0.0, scalar2=None, op0=ALU.max)
                nc.scalar.copy(out=V[:, c0:c0 + pn, :, 0:128],
                               in_=kvp[:, :pn, 256:512])
                nc.vector.scalar_tensor_tensor(
                    out=K[:, c0:c0 + pn, :], in0=e2[:, :pn, :], scalar=1.0,
                    in1=rr[:, :pn, :], op0=ALU.min, op1=ALU.add)
                if c0 + pn == NCH and last_rows < 128:
                    nc.vector.memset(K[last_rows:128, NCH - 1, :], 0.0)
                # stats matmuls trail by 2 pairs so the PE never waits on
                # the elu chain
                if pi >= 2:
                    emit_stats(*pairs[pi - 2])
            emit_stats(*pairs[-2])
            emit_stats(*pairs[-1])
            stats = stp.tile([128, 2, 129], F32, tag="stats")
            nc.vector.tensor_tensor(out=stats, in0=st[:, :, 0:129],
                                    in1=maskc, op=ALU.mult)

            if kind == "cross":
                nc.gpsimd.dma_start(out=cc_in[cross_idx][:, :],
                                    in_=stats.rearrange("p a b -> p (a b)"))
                nc.gpsimd.collective_compute(
                    "AllReduce", ALU.add,
                    ins=[cc_in[cross_idx][:, :]],
                    outs=[cc_out[cross_idx][:, :]],
                    replica_groups=groups)

            # ---- stage B second half (first half ran before stage A to
            # give the cross-layer AllReduce more PE work to hide under) ----
            for (s0, sw) in spans[5:]:
                emit_B(s0, sw)

            if kind == "cross":
                ssum = stp.tile([128, 2, 129], F32, tag="ssum")
                nc.gpsimd.dma_start(out=ssum.rearrange("p a b -> p (a b)"),
                                    in_=cc_out[cross_idx][:, :])
                pstats = stp.tile([128, 2, 129], F32, tag="pstats")
                nc.vector.tensor_sub(pstats, ssum, stats)
                cross_idx += 1
            else:
                pstats = stats

            kvbd = stp.tile([128, 2, 128], BF16, tag="kvbd")
            ksE = stp.tile([128, 2, 128], BF16, tag="ksE")
            nc.vector.memset(ksE, 0.0)
            for h in range(2):
                nc.vector.tensor_copy(out=kvbd[:, h, :], in_=pstats[:, h, 0:128])
                for b in range(4):
                    bs = slice(32 * b, 32 * (b + 1))
                    src = pstats[bs, h, 128:129]
                    src_b = bass.AP(tensor=src.tensor, offset=src.offset,
                                    ap=[src.ap[0], [0, 32]])
                    nc.vector.tensor_copy(out=ksE[bs, h, bs], in_=src_b)

            # ---- stage C: den + msg matmuls, msgT = msg / den ----
            msgT = big.tile([128, NCH, 2, 128], BF16, tag="big")
            for (s0, sw) in spans:
                for h in range(2):
                    dm = ps2.tile([128, 2, 512], F32, tag="big2")
                    nc.tensor.matmul(dm[:, 0, :sw], ksE[:, h, :],
                                     tsl(QT, h, s0, sw), start=True, stop=True)
                    nc.tensor.matmul(dm[:, 1, :sw], kvbd[:, h, :],
                                     tsl(QT, h, s0, sw), start=True, stop=True)
                    zr = sm.tile([128, 512], F32, tag="den", bufs=2)
                    nc.vector.reciprocal_approx_fast(out=zr[:, :sw],
                                                     in_=dm[:, 0, :sw])
                    nc.vector.tensor_tensor(out=tsl(msgT, h, s0, sw),
                                            in0=dm[:, 1, :sw], in1=zr[:, :sw],
                                            op=ALU.mult)

            # ---- stage D: merge natural + LN1 -> msgLN (4-chunk groups,
            # 2-chunk psum tiles + batched bn_stats / rstd ops) ----
            msgLN = big.tile([128, NCH, 256], BF16, tag="big")
            for g0 in range(0, NCH, 4):
                gn = min(4, NCH - g0)
                st6 = sm.tile([128, 4, 6], F32, tag="st6")
                mvb = sm.tile([128, 4, 2], F32, tag="mvb")
                md = ps2.tile([128, 4, 256], F32, tag="big2")
                for j in range(gn):
                    c = g0 + j
                    for t in range(2):
                        nc.tensor.matmul(md[:, j, :], msgT[:, c, t, :],
                                         wm[:, t, :],
                                         start=(t == 0), stop=(t == 1))
                    nc.vector.bn_stats(out=st6[:, j, :], in_=md[:, j, :])
                for j in range(gn):
                    nc.vector.bn_aggr(out=mvb[:, j, :], in_=st6[:, j, :])
                lnv = sm.tile([128, 4], F32, tag="lnv")
                nc.scalar.activation(out=lnv[:, :gn], in_=mvb[:, :gn, 1],
                                     func=AF.Ln, bias=epsc[:, :])
                rstd = sm.tile([128, 4], F32, tag="rstd")
                nc.scalar.activation(out=rstd[:, :gn], in_=lnv[:, :gn],
                                     func=AF.Exp, scale=-0.5)
                nmr = sm.tile([128, 4], F32, tag="nmr")
                nc.vector.scalar_tensor_tensor(out=nmr[:, :gn],
                                               in0=mvb[:, :gn, 0],
                                               scalar=-1.0, in1=rstd[:, :gn],
                                               op0=ALU.mult, op1=ALU.mult)
                for j in range(gn):
                    c = g0 + j
                    if j % 4 == 3:
                        nc.vector.tensor_scalar(
                            out=msgLN[:, c, :], in0=md[:, j, :],
                            scalar1=rstd[:, j:j + 1],
                            scalar2=nmr[:, j:j + 1],
                            op0=ALU.mult, op1=ALU.add)
                    else:
                        nc.scalar.activation(
                            out=msgLN[:, c, :], in_=md[:, j, :],
                            func=AF.Identity, scale=rstd[:, j:j + 1],
                            bias=nmr[:, j:j + 1])

            # ---- stage E: one batched DMA transpose msgLN -> msgLNT ----
            msgLNT = big.tile([128, NCH, 2, 128], BF16, tag="big")
            dma_T(msgLNT, msgLN)

            # ---- stages F+G, G deferred one span so its matmuls never
            # stall the PE on F's relu chain ----
            h2LN = big.tile([128, NCH, 256], BF16, tag="big")

            def emit_F(s0, sw):
                h1s = h1p.tile([128, 4, 512], BF16, tag="h1s")
                for m in range(4):
                    hps = psm.tile([128, 512], F32, tag="med")
                    for t in range(4):
                        rhs = (tsl(xT, t, s0, sw) if t < 2
                               else tsl(msgLNT, t - 2, s0, sw))
                        nc.tensor.matmul(hps[:, :sw],
                                         w1[:, t, 128 * m:128 * (m + 1)],
                                         rhs, start=(t == 0), stop=(t == 3))
                    if m == 3:
                        nc.vector.tensor_scalar(
                            out=h1s[:, m, :sw], in0=hps[:, :sw],
                            scalar1=b1s[:, m:m + 1], scalar2=0.0,
                            op0=ALU.add, op1=ALU.max)
                    else:
                        nc.scalar.activation(out=h1s[:, m, :sw],
                                             in_=hps[:, :sw], func=AF.Relu,
                                             bias=b1s[:, m:m + 1])
                return h1s

            def emit_G(s0, sw, h1s):
                ncl = sw // 128
                g2t = ps2.tile([128, 4, 256], F32, tag="big2")
                st6 = sm.tile([128, 4, 6], F32, tag="st6")
                mvb = sm.tile([128, 4, 2], F32, tag="mvb")
                for cl in range(ncl):
                    cls = slice(128 * cl, 128 * (cl + 1))
                    for t in range(4):
                        nc.tensor.matmul(g2t[:, cl, :], h1s[:, t, cls],
                                         w2[:, t, :],
                                         start=(t == 0), stop=(t == 3))
                for cl in range(ncl):
                    nc.vector.bn_stats(out=st6[:, cl, :], in_=g2t[:, cl, :])
                    nc.vector.bn_aggr(out=mvb[:, cl, :], in_=st6[:, cl, :])
                lnv = sm.tile([128, 4], F32, tag="lnv")
                nc.scalar.activation(out=lnv[:, :ncl], in_=mvb[:, :ncl, 1],
                                     func=AF.Ln, bias=epsc[:, :])
                rstd = sm.tile([128, 4], F32, tag="rstd")
                nc.scalar.activation(out=rstd[:, :ncl], in_=lnv[:, :ncl],
                                     func=AF.Exp, scale=-0.5)
                nmr = sm.tile([128, 4], F32, tag="nmr")
                nc.vector.scalar_tensor_tensor(out=nmr[:, :ncl],
                                               in0=mvb[:, :ncl, 0],
                                               scalar=-1.0, in1=rstd[:, :ncl],
                                               op0=ALU.mult, op1=ALU.mult)
                for cl in range(ncl):
                    c = s0 // 128 + cl
                    if g2b2_general:
                        hn_t = sm.tile([128, 256], F32, tag="hn")
                        nc.scalar.activation(out=hn_t, in_=g2t[:, cl, :],
                                             func=AF.Identity,
                                             scale=rstd[:, cl:cl + 1],
                                             bias=nmr[:, cl:cl + 1])
                        hg = sm.tile([128, 256], F32, tag="hg")
                        nc.vector.tensor_tensor(out=hg, in0=hn_t, in1=g2r,
                                                op=ALU.mult)
                        nc.vector.tensor_tensor(out=h2LN[:, c, :], in0=hg,
                                                in1=b2r, op=ALU.add)
                    else:
                        nc.scalar.activation(out=h2LN[:, c, :], in_=g2t[:, cl, :],
                                             func=AF.Identity,
                                             scale=rstd[:, cl:cl + 1],
                                             bias=nmr[:, cl:cl + 1])

            # H groups (transpose + residual add) interleave into the F/G
            # span loop; the small 2-chunk tail keeps the layer boundary off
            # the PE critical path.
            h2TT = big.tile([128, NCH, 2, 128], BF16, tag="big")
            xT_new = xtp.tile([128, NCH, 2, 128], BF16, tag="xT")
            h_groups = [(0, 10), (10, 10), (20, 10), (30, 6), (36, 2)]
            h_next = 0

            def emit_H(gi):
                g0, gn = h_groups[gi]
                gs = slice(g0, g0 + gn)
                nc.sync.dma_start_transpose(
                    out=h2TT[:, gs, :, :], in_=h2LN[:, gs, :])
                eng = nc.gpsimd if gi < 3 else nc.vector
                eng.tensor_tensor(out=xT_new[:, gs, :, :],
                                  in0=h2TT[:, gs, :, :],
                                  in1=xT[:, gs, :, :], op=ALU.add)

            prevF = None
            for (s0, sw) in spans:
                h1s = emit_F(s0, sw)
                if prevF is not None:
                    emit_G(*prevF)
                    done = prevF[0] // 128 + prevF[1] // 128
                    while (h_next < len(h_groups)
                           and h_groups[h_next][0] + h_groups[h_next][1] <= done):
                        emit_H(h_next)
                        h_next += 1
                prevF = (s0, sw, h1s)
            emit_G(*prevF)
            while h_next < len(h_groups):
                emit_H(h_next)
                h_next += 1
            xT = xT_new

        # ---- output: one DMA transpose back + cast f32 + DMA out ----
        onat = big.tile([128, NCH, 2, 128], BF16, tag="big")
        dma_Tinv(onat, xT)
        for c in range(0, NCH, 2):
            cn = min(2, NCH - c)
            onf = sm.tile([128, 2, 256], F32, tag="xin")
            nc.vector.tensor_copy(out=onf[:, :cn, :], in_=onat[:, c:c + cn, :, :])
            for j in range(cn):
                rows = last_rows if c + j == NCH - 1 else 128
                nc.sync.dma_start(out=out_d[128 * (c + j):128 * (c + j) + rows, :],
                                  in_=onf[:rows, j, :])

    nc.compile()
    return nc


L_SEQ = 4800
LAYER_KINDS = ("self", "cross", "self", "cross", "self", "cross", "self", "cross")

_cache = {}


def _get_program(weights):
    key = id(weights[0])
    if key not in _cache:
        _cache.clear()
        wmap, g2b2_general = prep_weights(*weights)
        prog = build(L_SEQ, LAYER_KINDS, g2b2_general)
        _cache[key] = (prog, wmap)
    return _cache[key]


def kernel(feat0, feat1, Wq, Wk, Wv, Wm, W1, W2, g1, b1, g2, b2):
    feat0 = np.asarray(feat0, dtype=np.float32)
    feat1 = np.asarray(feat1, dtype=np.float32)
    weights = tuple(np.asarray(w, dtype=np.float32)
                    for w in (Wq, Wk, Wv, Wm, W1, W2, g1, b1, g2, b2))
    prog, wmap = _get_program(weights)

    seqs = np.empty((N_CORES, L_SEQ, D_MODEL), np.float32)
    seqs[0::2] = feat0
    seqs[1::2] = feat1
    in_maps = [dict(x=np.ascontiguousarray(seqs[i]), **wmap)
               for i in range(N_CORES)]

    from concourse.bass_utils import run_bass_kernel_spmd
    res = run_bass_kernel_spmd(prog, in_maps, list(range(N_CORES)))
    out = np.stack([res.results[i]["out"] for i in range(N_CORES)])
    return out[0::2].copy(), out[1::2].copy()


# revision 26
# speedup vs baseline: 1.2277x; 1.2277x over previous
"""LoFTR LocalFeatureTransformer as a hand-written Bass/Tile SPMD kernel.

8 NeuronCores, one sequence per core (core i: feat{i%2}[i//2]); cross
layers exchange linear-attention KV statistics ([128,2,129] f32) with a
pairwise AllReduce (partner = sum - own).

The residual stream lives TRANSPOSED in SBUF in an interleaved layout
xT [128, NCH, 2, 128] bf16: element (p, c, t, l) = x[128*c + l,
128*t + p] — i.e. channel-within-half on partitions, then (l-chunk,
channel-half, l-within-chunk) along the free axis. This layout is what
one full-tensor DMA-xbar transpose of the natural [128, NCH, 256]
chunked layout produces, so natural<->transposed conversions are a
single hidden DMA op per tensor per layer instead of 38 PE transposes +
copies. LayerNorm runs in natural layout.

v2 changes (engine balance / instruction-count reduction):
- elu(x)+1 computed as max(x,0) + min(exp(x),1): exp on ACT straight
  from PSUM, relu on DVE, min+add as one scalar_tensor_tensor on
  GPSIMD (otherwise idle). No serial RELU->EXP chain on one engine.
- K/V projection runs on 2-chunk PSUM pairs ([128,2,512], 2 banks) so
  every elementwise op covers 512 free elements - the ~150-250 ns
  fixed cost per ACT/DVE instruction is paid half as often.
- Q projection per span in one [128,2,512] PSUM tile; elu ops batched
  over both output halves ([128,1024] free).
- attention Z: single tensor_tensor divide (msg/den) instead of
  reciprocal_approx + multiply.
- LN stats via 2-chunk-batched bn_stats ([128,2,256] -> [128,2,6]);
  ln/exp/nmr small-ops batched per 4-chunk group; LN applies rotated
  ACT:DVE 3:1.
- stats matmuls interleaved into the K/V pair loop so the PE queue
  stays dense (HAM stays at K=8/8).
- dummy AllReduce warm-up during layer 0 to pay collective setup cost
  off the critical path.

All matmuls bf16 / fp32 PSUM. LN1 gamma/beta folded exactly into W1's
msg half + bias; LN2 general path only when gamma/beta not ones/zeros.
The activation-table pass is pinned to natural_log_exp_and_others
(covers exp/ln/relu/copy/identity) — the stock greedy pass thrashes
1.28us table reloads between exp and ln tables.
"""

from contextlib import ExitStack

import numpy as np
import ml_dtypes

import concourse.bass as bass
import concourse.mybir as mybir
import concourse.tile as tile
from concourse import bacc
from concourse.hw_specs import get_activation_tables

F32 = mybir.dt.float32
BF16 = mybir.dt.bfloat16
AF = mybir.ActivationFunctionType
ALU = mybir.AluOpType

D_MODEL = 256
LN_EPS = 1e-5
N_CORES = 8


class _BaccOneTable(bacc.Bacc):
    ACT_TABLE = "natural_log_exp_and_others"

    def insert_act_table_loads(self):
        has_activation = any(
            isinstance(i, mybir.InstActivation)
            for b in self.main_func.blocks
            for i in b.instructions)
        if not has_activation:
            return
        tables = [(n, (s if n == self.ACT_TABLE else set()))
                  for n, s in get_activation_tables(self.m.arch).items()]
        bacc._bass_rust.insert_act_table_loads(self, tables)


def _spans(LP, step=512):
    return [(s, min(step, LP - s)) for s in range(0, LP, step)]


def prep_weights(Wq, Wk, Wv, Wm, W1, W2, g1, b1, g2, b2):
    bf = ml_dtypes.bfloat16

    def tile_w(w):
        nl, fi, fo = w.shape
        return np.ascontiguousarray(
            w.reshape(nl, fi // 128, 128, fo)).astype(bf)

    W1f = np.array(W1, dtype=np.float32, copy=True)
    W1f[:, D_MODEL:, :] *= g1[:, :, None]
    bias1 = np.einsum("lc,lco->lo", b1, W1[:, D_MODEL:, :]).astype(np.float32)
    g2b2_general = not (np.allclose(g2, 1.0) and np.allclose(b2, 0.0))

    mask = np.zeros((128, 2, 129), dtype=bf)
    for b in range(4):
        mask[32 * b:32 * (b + 1), :, 32 * b:32 * (b + 1)] = 1.0
    mask[:, :, 128] = 1.0

    Wkv = np.concatenate([Wk, Wv], axis=2)  # [NL, 256, 512]
    wmap = {
        "wq": tile_w(Wq), "wkv": tile_w(Wkv), "wm": tile_w(Wm),
        "w1": tile_w(W1f), "w2": tile_w(W2),
        "bias1": bias1, "maskc": mask,
    }
    if g2b2_general:
        wmap["g2"] = g2.astype(np.float32)
        wmap["b2"] = b2.astype(np.float32)
    return wmap, g2b2_general


def build(L, kinds, g2b2_general):
    NL = len(kinds)
    LP = -(-L // 128) * 128
    NCH = LP // 128
    last_rows = L - 128 * (NCH - 1)
    spans = _spans(LP)

    nc = _BaccOneTable()
    x_in = nc.declare_dram_parameter("x", [L, D_MODEL], F32, isOutput=False)
    wq_in = nc.declare_dram_parameter("wq", [NL, 2, 128, 256], BF16, isOutput=False)
    wkv_in = nc.declare_dram_parameter("wkv", [NL, 2, 128, 512], BF16, isOutput=False)
    wm_in = nc.declare_dram_parameter("wm", [NL, 2, 128, 256], BF16, isOutput=False)
    w1_in = nc.declare_dram_parameter("w1", [NL, 4, 128, 512], BF16, isOutput=False)
    w2_in = nc.declare_dram_parameter("w2", [NL, 4, 128, 256], BF16, isOutput=False)
    b1_in = nc.declare_dram_parameter("bias1", [NL, 512], F32, isOutput=False)
    mk_in = nc.declare_dram_parameter("maskc", [128, 2, 129], BF16, isOutput=False)
    if g2b2_general:
        g2_in = nc.declare_dram_parameter("g2", [NL, 256], F32, isOutput=False)
        b2_in = nc.declare_dram_parameter("b2", [NL, 256], F32, isOutput=False)
    out_d = nc.declare_dram_parameter("out", [L, D_MODEL], F32, isOutput=True)

    n_cross = sum(1 for k in kinds if k == "cross")
    cc_in = [nc.dram_tensor(f"cc_in{i}", [128, 258], F32) for i in range(n_cross)]
    cc_out = [nc.dram_tensor(f"cc_out{i}", [128, 258], F32) for i in range(n_cross)]
    ccw_in = nc.dram_tensor("ccw_in", [128, 8], F32)
    ccw_out = nc.dram_tensor("ccw_out", [128, 8], F32)
    groups = [[2 * i, 2 * i + 1] for i in range(N_CORES // 2)]

    with ExitStack() as ctx:
        tc = ctx.enter_context(tile.TileContext(nc))
        cons = ctx.enter_context(tc.tile_pool(name="cons", bufs=1))
        wpool = ctx.enter_context(tc.tile_pool(name="wts", bufs=2))
        xtp = ctx.enter_context(tc.tile_pool(name="xtp", bufs=2))
        qp = ctx.enter_context(tc.tile_pool(name="qp", bufs=1))
        big = ctx.enter_context(tc.tile_pool(name="big", bufs=3))
        h1p = ctx.enter_context(tc.tile_pool(name="h1p", bufs=2))
        sm = ctx.enter_context(tc.tile_pool(name="sm", bufs=2))
        stp = ctx.enter_context(tc.tile_pool(name="stp", bufs=2))
        # PSUM: 8 banks total = ps2 2x[128,1024] (4) + psm 2x[128,512] (2)
        # + psst 1x[128,2,512] (2; one bank per stats half -- a start=True
        # matmul clears the whole bank's has_written bits, so the two
        # interleaved h-accumulations must not share a bank)
        ps2 = ctx.enter_context(tc.tile_pool(name="ps2", bufs=2, space="PSUM"))
        psm = ctx.enter_context(tc.tile_pool(name="psm", bufs=2, space="PSUM"))
        psst = ctx.enter_context(tc.tile_pool(name="psst", bufs=1, space="PSUM"))

        maskc = cons.tile([128, 2, 129], BF16)
        nc.sync.dma_start(out=maskc, in_=mk_in[:, :, :])
        epsc = cons.tile([128, 1], F32)
        nc.vector.memset(epsc, LN_EPS)

        # rhs AP for (t, span) of an interleaved transposed tensor
        def tsl(xt, t, s0, sw):
            return xt[:, s0 // 128:(s0 + sw) // 128, t, :]

        def dma_T(dst, srcn):
            # grouped transpose: src natural [128, NCH, 256] -> dst interleaved
            for g0 in range(0, NCH, 10):
                gn = min(10, NCH - g0)
                nc.sync.dma_start_transpose(
                    out=dst[:, g0:g0 + gn, :, :], in_=srcn[:, g0:g0 + gn, :])

        def dma_Tinv(dstn, src):
            # grouped transpose: src interleaved -> dst natural
            for g0 in range(0, NCH, 10):
                gn = min(10, NCH - g0)
                nc.sync.dma_start_transpose(
                    out=dstn[:, g0:g0 + gn, :, :], in_=src[:, g0:g0 + gn, :, :])

        # ---- load input, cast bf16, one batched transpose -> xT ----
        xnat = big.tile([128, NCH, 256], BF16, tag="big")
        for c in range(0, NCH, 2):
            cn = min(2, NCH - c)
            xin = sm.tile([128, 2, 256], F32, tag="xin")
            for j in range(cn):
                rows = last_rows if c + j == NCH - 1 else 128
                if rows < 128:
                    nc.vector.memset(xin[:, j, :], 0.0)
                nc.sync.dma_start(out=xin[:rows, j, :],
                                  in_=x_in[128 * (c + j):128 * (c + j) + rows, :])
            nc.vector.tensor_copy(out=xnat[:, c:c + cn, :], in_=xin[:, :cn, :])
        xT = xtp.tile([128, NCH, 2, 128], BF16, tag="xT")
        dma_T(xT, xnat)

        # warm up the collective path off the critical path (layer 0 is
        # 'self'; first real AllReduce is layer 1)
        ccw_t = sm.tile([128, 8], F32, tag="ccw")
        nc.vector.memset(ccw_t, 0.0)
        nc.gpsimd.dma_start(out=ccw_in[:, :], in_=ccw_t)
        nc.gpsimd.collective_compute(
            "AllReduce", ALU.add, ins=[ccw_in[:, :]], outs=[ccw_out[:, :]],
            replica_groups=groups)
        ccw_r = sm.tile([128, 8], F32, tag="ccwr")
        nc.gpsimd.dma_start(out=ccw_r, in_=ccw_out[:, :])

        cross_idx = 0
        for li, kind in enumerate(kinds):
            wq = wpool.tile([128, 2, 256], BF16, tag="wq")
            wkv = wpool.tile([128, 2, 512], BF16, tag="wkv")
            wm = wpool.tile([128, 2, 256], BF16, tag="wm")
            w1 = wpool.tile([128, 4, 512], BF16, tag="w1")
            w2 = wpool.tile([128, 4, 256], BF16, tag="w2")
            b1s = wpool.tile([128, 4], F32, tag="b1s")
            for sb_t, dr in ((wq, wq_in), (wkv, wkv_in), (wm, wm_in),
                             (w1, w1_in), (w2, w2_in)):
                nc.sync.dma_start(out=sb_t, in_=dr[li].rearrange("t p n -> p t n"))
            nc.sync.dma_start(out=b1s, in_=b1_in[li].rearrange("(m p) -> p m", p=128))
            if g2b2_general:
                g2r = wpool.tile([128, 256], F32, tag="g2r")
                b2r = wpool.tile([128, 256], F32, tag="b2r")
                nc.sync.dma_start(out=g2r,
                                  in_=g2_in[li:li + 1, :].to_broadcast((128, 256)))
                nc.sync.dma_start(out=b2r,
                                  in_=b2_in[li:li + 1, :].to_broadcast((128, 256)))

            # ---- stage B (first half): q^T + elu -> QT ----
            QT = qp.tile([128, NCH, 2, 128], BF16, tag="QT")

            def emit_B(s0, sw):
                ncl = sw // 128
                qp2 = ps2.tile([128, 2, 512], F32, tag="big2")
                for m in range(2):
                    for t in range(2):
                        nc.tensor.matmul(qp2[:, m, :sw],
                                         wq[:, t, 128 * m:128 * (m + 1)],
                                         tsl(xT, t, s0, sw),
                                         start=(t == 0), stop=(t == 1))
                e2 = sm.tile([128, 2, 512], F32, tag="eQ", bufs=2)
                nc.scalar.activation(out=e2[:, :, :sw], in_=qp2[:, :, :sw],
                                     func=AF.Exp)
                rr = sm.tile([128, 2, 512], F32, tag="rQ", bufs=2)
                nc.scalar.activation(out=rr[:, 0, :sw], in_=qp2[:, 0, :sw],
                                     func=AF.Relu)
                nc.vector.tensor_scalar(out=rr[:, 1, :sw], in0=qp2[:, 1, :sw],
                                        scalar1=0.0, scalar2=None, op0=ALU.max)
                for m in range(2):
                    nc.vector.scalar_tensor_tensor(
                        out=QT[:, s0 // 128:s0 // 128 + ncl, m, :],
                        in0=e2[:, m, :sw], scalar=1.0,
                        in1=rr[:, m, :sw], op0=ALU.min, op1=ALU.add)

            for (s0, sw) in spans[:5]:
                emit_B(s0, sw)

            # ---- stage A: k|v fused matmul on 2-chunk pairs; elu(K);
            # stats interleaved.  elu(x)+1 = min(exp(x),1) + max(x,0) ----
            K = big.tile([128, NCH, 256], BF16, tag="big")
            V = big.tile([128, NCH, 2, 129], BF16, tag="big")
            nc.vector.memset(V[:, :, :, 128:129], 1.0)
            st = psst.tile([128, 2, 512], F32, tag="st")

            def emit_stats(c0, pn):
                for j in range(pn):
                    c = c0 + j
                    for h in range(2):
                        nc.tensor.matmul(st[:, h, 0:129],
                                         K[:, c, 128 * h:128 * (h + 1)],
                                         V[:, c, h, :],
                                         start=(c == 0), stop=(c == NCH - 1))

            pairs = [(c0, min(2, NCH - c0)) for c0 in range(0, NCH, 2)]
            for pi, (c0, pn) in enumerate(pairs):
                kvp = ps2.tile([128, 2, 512], F32, tag="big2")
                for j in range(pn):
                    for t in range(2):
                        nc.tensor.matmul(kvp[:, j, :], xT[:, c0 + j, t, :],
                                         wkv[:, t, :],
                                         start=(t == 0), stop=(t == 1))
                kk = kvp[:, :pn, 0:256]
                # elu(k)+1 = min(exp(k),1) + max(k,0); f32 intermediates.
                # exp and relu both read PSUM directly (slot frees after 2
                # parallel hops), one stt combines from SBUF.
                e2 = sm.tile([128, 2, 256], F32, tag="eK", bufs=2)
                nc.scalar.activation(out=e2[:, :pn, :], in_=kk, func=AF.Exp)
                rr = sm.tile([128, 2, 256], F32, tag="rK", bufs=2)
                nc.vector.tensor_scalar(out=rr[:, :pn, :], in0=kk,
                                        scalar1=0.0, scalar2=None, op0=ALU.max)
                nc.scalar.copy(out=V[:, c0:c0 + pn, :, 0:128],
                               in_=kvp[:, :pn, 256:512])
                nc.vector.scalar_tensor_tensor(
                    out=K[:, c0:c0 + pn, :], in0=e2[:, :pn, :], scalar=1.0,
                    in1=rr[:, :pn, :], op0=ALU.min, op1=ALU.add)
                if c0 + pn == NCH and last_rows < 128:
                    nc.vector.memset(K[last_rows:128, NCH - 1, :], 0.0)
                # stats matmuls trail by 2 pairs so the PE never waits on
                # the elu chain
                if pi >= 2:
                    emit_stats(*pairs[pi - 2])
            emit_stats(*pairs[-2])
            emit_stats(*pairs[-1])
            stats = stp.tile([128, 2, 129], F32, tag="stats")
            nc.vector.tensor_tensor(out=stats, in0=st[:, :, 0:129],
                                    in1=maskc, op=ALU.mult)

            if kind == "cross":
                nc.gpsimd.dma_start(out=cc_in[cross_idx][:, :],
                                    in_=stats.rearrange("p a b -> p (a b)"))
                nc.gpsimd.collective_compute(
                    "AllReduce", ALU.add,
                    ins=[cc_in[cross_idx][:, :]],
                    outs=[cc_out[cross_idx][:, :]],
                    replica_groups=groups)

            # ---- stage B second half (first half ran before stage A to
            # give the cross-layer AllReduce more PE work to hide under) ----
            for (s0, sw) in spans[5:]:
                emit_B(s0, sw)

            if kind == "cross":
                ssum = stp.tile([128, 2, 129], F32, tag="ssum")
                nc.gpsimd.dma_start(out=ssum.rearrange("p a b -> p (a b)"),
                                    in_=cc_out[cross_idx][:, :])
                pstats = stp.tile([128, 2, 129], F32, tag="pstats")
                nc.vector.tensor_sub(pstats, ssum, stats)
                cross_idx += 1
            else:
                pstats = stats

            kvbd = stp.tile([128, 2, 128], BF16, tag="kvbd")
            ksE = stp.tile([128, 2, 128], BF16, tag="ksE")
            nc.vector.memset(ksE, 0.0)
            for h in range(2):
                nc.vector.tensor_copy(out=kvbd[:, h, :], in_=pstats[:, h, 0:128])
                for b in range(4):
                    bs = slice(32 * b, 32 * (b + 1))
                    src = pstats[bs, h, 128:129]
                    src_b = bass.AP(tensor=src.tensor, offset=src.offset,
                                    ap=[src.ap[0], [0, 32]])
                    nc.vector.tensor_copy(out=ksE[bs, h, bs], in_=src_b)

            # ---- stage C: den + msg matmuls, msgT = msg / den ----
            msgT = big.tile([128, NCH, 2, 128], BF16, tag="big")
            for (s0, sw) in spans:
                for h in range(2):
                    dm = ps2.tile([128, 2, 512], F32, tag="big2")
                    nc.tensor.matmul(dm[:, 0, :sw], ksE[:, h, :],
                                     tsl(QT, h, s0, sw), start=True, stop=True)
                    nc.tensor.matmul(dm[:, 1, :sw], kvbd[:, h, :],
                                     tsl(QT, h, s0, sw), start=True, stop=True)
                    zr = sm.tile([128, 512], F32, tag="den", bufs=2)
                    nc.vector.reciprocal_approx_fast(out=zr[:, :sw],
                                                     in_=dm[:, 0, :sw])
                    nc.vector.tensor_tensor(out=tsl(msgT, h, s0, sw),
                                            in0=dm[:, 1, :sw], in1=zr[:, :sw],
                                            op=ALU.mult)

            # ---- stage D: merge natural + LN1 -> msgLN (4-chunk groups,
            # 2-chunk psum tiles + batched bn_stats / rstd ops) ----
            msgLN = big.tile([128, NCH, 256], BF16, tag="big")
            for g0 in range(0, NCH, 4):
                gn = min(4, NCH - g0)
                st6 = sm.tile([128, 4, 6], F32, tag="st6")
                mvb = sm.tile([128, 4, 2], F32, tag="mvb")
                md = ps2.tile([128, 4, 256], F32, tag="big2")
                for j in range(gn):
                    c = g0 + j
                    for t in range(2):
                        nc.tensor.matmul(md[:, j, :], msgT[:, c, t, :],
                                         wm[:, t, :],
                                         start=(t == 0), stop=(t == 1))
                    nc.vector.bn_stats(out=st6[:, j, :], in_=md[:, j, :])
                for j in range(gn):
                    nc.vector.bn_aggr(out=mvb[:, j, :], in_=st6[:, j, :])
                lnv = sm.tile([128, 4], F32, tag="lnv")
                nc.scalar.activation(out=lnv[:, :gn], in_=mvb[:, :gn, 1],
                                     func=AF.Ln, bias=epsc[:, :])
                rstd = sm.tile([128, 4], F32, tag="rstd")
                nc.scalar.activation(out=rstd[:, :gn], in_=lnv[:, :gn],
                                     func=AF.Exp, scale=-0.5)
                nmr = sm.tile([128, 4], F32, tag="nmr")
                nc.vector.scalar_tensor_tensor(out=nmr[:, :gn],
                                               in0=mvb[:, :gn, 0],
                                               scalar=-1.0, in1=rstd[:, :gn],
                                               op0=ALU.mult, op1=ALU.mult)
                for j in range(gn):
                    c = g0 + j
                    if j % 4 == 3:
                        nc.vector.tensor_scalar(
                            out=msgLN[:, c, :], in0=md[:, j, :],
                            scalar1=rstd[:, j:j + 1],
                            scalar2=nmr[:, j:j + 1],
                            op0=ALU.mult, op1=ALU.add)
                    else:
                        nc.scalar.activation(
                            out=msgLN[:, c, :], in_=md[:, j, :],
                            func=AF.Identity, scale=rstd[:, j:j + 1],
                            bias=nmr[:, j:j + 1])

            # ---- stage E: one batched DMA transpose msgLN -> msgLNT ----
            msgLNT = big.tile([128, NCH, 2, 128], BF16, tag="big")
            dma_T(msgLNT, msgLN)

            # ---- stages F+G, G deferred one span so its matmuls never
            # stall the PE on F's relu chain ----
            h2LN = big.tile([128, NCH, 256], BF16, tag="big")

            def emit_F(s0, sw):
                h1s = h1p.tile([128, 4, 512], BF16, tag="h1s")
                for m in range(4):
                    hps = psm.tile([128, 512], F32, tag="med")
                    for t in range(4):
                        rhs = (tsl(xT, t, s0, sw) if t < 2
                               else tsl(msgLNT, t - 2, s0, sw))
                        nc.tensor.matmul(hps[:, :sw],
                                         w1[:, t, 128 * m:128 * (m + 1)],
                                         rhs, start=(t == 0), stop=(t == 3))
                    if m == 3:
                        nc.vector.tensor_scalar(
                            out=h1s[:, m, :sw], in0=hps[:, :sw],
                            scalar1=b1s[:, m:m + 1], scalar2=0.0,
                            op0=ALU.add, op1=ALU.max)
                    else:
                        nc.scalar.activation(out=h1s[:, m, :sw],
                                             in_=hps[:, :sw], func=AF.Relu,
                                             bias=b1s[:, m:m + 1])
                return h1s

            def emit_G(s0, sw, h1s):
                ncl = sw // 128
                g2t = ps2.tile([128, 4, 256], F32, tag="big2")
                st6 = sm.tile([128, 4, 6], F32, tag="st6")
                mvb = sm.tile([128, 4, 2], F32, tag="mvb")
                for cl in range(ncl):
                    cls = slice(128 * cl, 128 * (cl + 1))
                    for t in range(4):
                        nc.tensor.matmul(g2t[:, cl, :], h1s[:, t, cls],
                                         w2[:, t, :],
                                         start=(t == 0), stop=(t == 3))
                for cl in range(ncl):
                    nc.vector.bn_stats(out=st6[:, cl, :], in_=g2t[:, cl, :])
                    nc.vector.bn_aggr(out=mvb[:, cl, :], in_=st6[:, cl, :])
                lnv = sm.tile([128, 4], F32, tag="lnv")
                nc.scalar.activation(out=lnv[:, :ncl], in_=mvb[:, :ncl, 1],
                                     func=AF.Ln, bias=epsc[:, :])
                rstd = sm.tile([128, 4], F32, tag="rstd")
                nc.scalar.activation(out=rstd[:, :ncl], in_=lnv[:, :ncl],
                                     func=AF.Exp, scale=-0.5)
                nmr = sm.tile([128, 4], F32, tag="nmr")
                nc.vector.scalar_tensor_tensor(out=nmr[:, :ncl],
                                               in0=mvb[:, :ncl, 0],
                                               scalar=-1.0, in1=rstd[:, :ncl],
                                               op0=ALU.mult, op1=ALU.mult)
                for cl in range(ncl):
                    c = s0 // 128 + cl
                    if g2b2_general:
                        hn_t = sm.tile([128, 256], F32, tag="hn")
                        nc.scalar.activation(out=hn_t, in_=g2t[:, cl, :],
                                             func=AF.Identity,
                                             scale=rstd[:, cl:cl + 1],
                                             bias=nmr[:, cl:cl + 1])
                        hg = sm.tile([128, 256], F32, tag="hg")
                        nc.vector.tensor_tensor(out=hg, in0=hn_t, in1=g2r,
                                                op=ALU.mult)
                        nc.vector.tensor_tensor(out=h2LN[:, c, :], in0=hg,
                                                in1=b2r, op=ALU.add)
                    else:
                        nc.scalar.activation(out=h2LN[:, c, :], in_=g2t[:, cl, :],
                                             func=AF.Identity,
                                             scale=rstd[:, cl:cl + 1],
                                             bias=nmr[:, cl:cl + 1])

            # H groups (transpose + residual add) interleave into the F/G
            # span loop; the small 2-chunk tail keeps the layer boundary off
            # the PE critical path.
            h2TT = big.tile([128, NCH, 2, 128], BF16, tag="big")
            xT_new = xtp.tile([128, NCH, 2, 128], BF16, tag="xT")
            h_groups = [(0, 10), (10, 10), (20, 10), (30, 6), (36, 2)]
            h_next = 0

            def emit_H(gi):
                g0, gn = h_groups[gi]
                gs = slice(g0, g0 + gn)
                nc.sync.dma_start_transpose(
                    out=h2TT[:, gs, :, :], in_=h2LN[:, gs, :])
                eng = nc.vector
                eng.tensor_tensor(out=xT_new[:, gs, :, :],
                                  in0=h2TT[:, gs, :, :],
                                  in1=xT[:, gs, :, :], op=ALU.add)

            prevF = None
            for (s0, sw) in spans:
                h1s = emit_F(s0, sw)
                if prevF is not None:
                    emit_G(*prevF)
                    done = prevF[0] // 128 + prevF[1] // 128
                    while (h_next < len(h_groups)
                           and h_groups[h_next][0] + h_groups[h_next][1] <= done):
                        emit_H(h_next)
                        h_next += 1
                prevF = (s0, sw, h1s)
            emit_G(*prevF)
            while h_next < len(h_groups):
                emit_H(h_next)
                h_next += 1
            xT = xT_new

        # ---- output: one DMA transpose back + cast f32 + DMA out ----
        onat = big.tile([128, NCH, 2, 128], BF16, tag="big")
        dma_Tinv(onat, xT)
        for c in range(0, NCH, 2):
            cn = min(2, NCH - c)
            onf = sm.tile([128, 2, 256], F32, tag="xin")
            nc.vector.tensor_copy(out=onf[:, :cn, :], in_=onat[:, c:c + cn, :, :])
            for j in range(cn):
                rows = last_rows if c + j == NCH - 1 else 128
                nc.sync.dma_start(out=out_d[128 * (c + j):128 * (c + j) + rows, :],
                                  in_=onf[:rows, j, :])

    nc.compile()
    return nc


L_SEQ = 4800
LAYER_KINDS = ("self", "cross", "self", "cross", "self", "cross", "self", "cross")

_cache = {}


def _get_program(weights):
    key = id(weights[0])
    if key not in _cache:
        _cache.clear()
        wmap, g2b2_general = prep_weights(*weights)
        prog = build(L_SEQ, LAYER_KINDS, g2b2_general)
        _cache[key] = (prog, wmap)
    return _cache[key]


def kernel(feat0, feat1, Wq, Wk, Wv, Wm, W1, W2, g1, b1, g2, b2):
    feat0 = np.asarray(feat0, dtype=np.float32)
    feat1 = np.asarray(feat1, dtype=np.float32)
    weights = tuple(np.asarray(w, dtype=np.float32)
                    for w in (Wq, Wk, Wv, Wm, W1, W2, g1, b1, g2, b2))
    prog, wmap = _get_program(weights)

    seqs = np.empty((N_CORES, L_SEQ, D_MODEL), np.float32)
    seqs[0::2] = feat0
    seqs[1::2] = feat1
    in_maps = [dict(x=np.ascontiguousarray(seqs[i]), **wmap)
               for i in range(N_CORES)]

    from concourse.bass_utils import run_bass_kernel_spmd
    res = run_bass_kernel_spmd(prog, in_maps, list(range(N_CORES)))
    out = np.stack([res.results[i]["out"] for i in range(N_CORES)])
    return out[0::2].copy(), out[1::2].copy()


# revision 27
# speedup vs baseline: 1.2725x; 1.0364x over previous
"""LoFTR LocalFeatureTransformer as a hand-written Bass/Tile SPMD kernel.

8 NeuronCores, one sequence per core (core i: feat{i%2}[i//2]); cross
layers exchange linear-attention KV statistics ([128,2,129] f32) with a
pairwise AllReduce (partner = sum - own).

The residual stream lives TRANSPOSED in SBUF in an interleaved layout
xT [128, NCH, 2, 128] bf16: element (p, c, t, l) = x[128*c + l,
128*t + p] — i.e. channel-within-half on partitions, then (l-chunk,
channel-half, l-within-chunk) along the free axis. This layout is what
one full-tensor DMA-xbar transpose of the natural [128, NCH, 256]
chunked layout produces, so natural<->transposed conversions are a
single hidden DMA op per tensor per layer instead of 38 PE transposes +
copies. LayerNorm runs in natural layout.

v2 changes (engine balance / instruction-count reduction):
- elu(x)+1 computed as max(x,0) + min(exp(x),1): exp on ACT straight
  from PSUM, relu on DVE, min+add as one scalar_tensor_tensor on
  GPSIMD (otherwise idle). No serial RELU->EXP chain on one engine.
- K/V projection runs on 2-chunk PSUM pairs ([128,2,512], 2 banks) so
  every elementwise op covers 512 free elements - the ~150-250 ns
  fixed cost per ACT/DVE instruction is paid half as often.
- Q projection per span in one [128,2,512] PSUM tile; elu ops batched
  over both output halves ([128,1024] free).
- attention Z: single tensor_tensor divide (msg/den) instead of
  reciprocal_approx + multiply.
- LN stats via 2-chunk-batched bn_stats ([128,2,256] -> [128,2,6]);
  ln/exp/nmr small-ops batched per 4-chunk group; LN applies rotated
  ACT:DVE 3:1.
- stats matmuls interleaved into the K/V pair loop so the PE queue
  stays dense (HAM stays at K=8/8).
- dummy AllReduce warm-up during layer 0 to pay collective setup cost
  off the critical path.

All matmuls bf16 / fp32 PSUM. LN1 gamma/beta folded exactly into W1's
msg half + bias; LN2 general path only when gamma/beta not ones/zeros.
The activation-table pass is pinned to natural_log_exp_and_others
(covers exp/ln/relu/copy/identity) — the stock greedy pass thrashes
1.28us table reloads between exp and ln tables.
"""

from contextlib import ExitStack

import numpy as np
import ml_dtypes

import concourse.bass as bass
import concourse.mybir as mybir
import concourse.tile as tile
from concourse import bacc
from concourse.hw_specs import get_activation_tables

F32 = mybir.dt.float32
BF16 = mybir.dt.bfloat16
AF = mybir.ActivationFunctionType
ALU = mybir.AluOpType

D_MODEL = 256
LN_EPS = 1e-5
N_CORES = 8


class _BaccOneTable(bacc.Bacc):
    ACT_TABLE = "natural_log_exp_and_others"

    def insert_act_table_loads(self):
        has_activation = any(
            isinstance(i, mybir.InstActivation)
            for b in self.main_func.blocks
            for i in b.instructions)
        if not has_activation:
            return
        tables = [(n, (s if n == self.ACT_TABLE else set()))
                  for n, s in get_activation_tables(self.m.arch).items()]
        bacc._bass_rust.insert_act_table_loads(self, tables)


def _spans(LP, step=512):
    return [(s, min(step, LP - s)) for s in range(0, LP, step)]


def prep_weights(Wq, Wk, Wv, Wm, W1, W2, g1, b1, g2, b2):
    bf = ml_dtypes.bfloat16

    def tile_w(w):
        nl, fi, fo = w.shape
        return np.ascontiguousarray(
            w.reshape(nl, fi // 128, 128, fo)).astype(bf)

    W1f = np.array(W1, dtype=np.float32, copy=True)
    W1f[:, D_MODEL:, :] *= g1[:, :, None]
    bias1 = np.einsum("lc,lco->lo", b1, W1[:, D_MODEL:, :]).astype(np.float32)
    g2b2_general = not (np.allclose(g2, 1.0) and np.allclose(b2, 0.0))

    mask = np.zeros((128, 2, 129), dtype=bf)
    for b in range(4):
        mask[32 * b:32 * (b + 1), :, 32 * b:32 * (b + 1)] = 1.0
    mask[:, :, 128] = 1.0

    Wkv = np.concatenate([Wk, Wv], axis=2)  # [NL, 256, 512]
    wmap = {
        "wq": tile_w(Wq), "wkv": tile_w(Wkv), "wm": tile_w(Wm),
        "w1": tile_w(W1f), "w2": tile_w(W2),
        "bias1": bias1, "maskc": mask,
    }
    if g2b2_general:
        wmap["g2"] = g2.astype(np.float32)
        wmap["b2"] = b2.astype(np.float32)
    return wmap, g2b2_general


def build(L, kinds, g2b2_general):
    NL = len(kinds)
    LP = -(-L // 128) * 128
    NCH = LP // 128
    last_rows = L - 128 * (NCH - 1)
    spans = _spans(LP)

    nc = _BaccOneTable()
    x_in = nc.declare_dram_parameter("x", [L, D_MODEL], F32, isOutput=False)
    wq_in = nc.declare_dram_parameter("wq", [NL, 2, 128, 256], BF16, isOutput=False)
    wkv_in = nc.declare_dram_parameter("wkv", [NL, 2, 128, 512], BF16, isOutput=False)
    wm_in = nc.declare_dram_parameter("wm", [NL, 2, 128, 256], BF16, isOutput=False)
    w1_in = nc.declare_dram_parameter("w1", [NL, 4, 128, 512], BF16, isOutput=False)
    w2_in = nc.declare_dram_parameter("w2", [NL, 4, 128, 256], BF16, isOutput=False)
    b1_in = nc.declare_dram_parameter("bias1", [NL, 512], F32, isOutput=False)
    mk_in = nc.declare_dram_parameter("maskc", [128, 2, 129], BF16, isOutput=False)
    if g2b2_general:
        g2_in = nc.declare_dram_parameter("g2", [NL, 256], F32, isOutput=False)
        b2_in = nc.declare_dram_parameter("b2", [NL, 256], F32, isOutput=False)
    out_d = nc.declare_dram_parameter("out", [L, D_MODEL], F32, isOutput=True)

    n_cross = sum(1 for k in kinds if k == "cross")
    cc_in = [nc.dram_tensor(f"cc_in{i}", [128, 258], F32) for i in range(n_cross)]
    cc_out = [nc.dram_tensor(f"cc_out{i}", [128, 258], F32) for i in range(n_cross)]
    ccw_in = nc.dram_tensor("ccw_in", [128, 8], F32)
    ccw_out = nc.dram_tensor("ccw_out", [128, 8], F32)
    groups = [[2 * i, 2 * i + 1] for i in range(N_CORES // 2)]

    with ExitStack() as ctx:
        tc = ctx.enter_context(tile.TileContext(nc))
        cons = ctx.enter_context(tc.tile_pool(name="cons", bufs=1))
        wpool = ctx.enter_context(tc.tile_pool(name="wts", bufs=2))
        xtp = ctx.enter_context(tc.tile_pool(name="xtp", bufs=2))
        qp = ctx.enter_context(tc.tile_pool(name="qp", bufs=1))
        big = ctx.enter_context(tc.tile_pool(name="big", bufs=3))
        h1p = ctx.enter_context(tc.tile_pool(name="h1p", bufs=2))
        sm = ctx.enter_context(tc.tile_pool(name="sm", bufs=2))
        stp = ctx.enter_context(tc.tile_pool(name="stp", bufs=2))
        # PSUM: 8 banks total = ps2 2x[128,1024] (4) + psm 2x[128,512] (2)
        # + psst 1x[128,2,512] (2; one bank per stats half -- a start=True
        # matmul clears the whole bank's has_written bits, so the two
        # interleaved h-accumulations must not share a bank)
        ps2 = ctx.enter_context(tc.tile_pool(name="ps2", bufs=2, space="PSUM"))
        psm = ctx.enter_context(tc.tile_pool(name="psm", bufs=2, space="PSUM"))
        psst = ctx.enter_context(tc.tile_pool(name="psst", bufs=1, space="PSUM"))

        maskc = cons.tile([128, 2, 129], BF16)
        nc.sync.dma_start(out=maskc, in_=mk_in[:, :, :])
        epsc = cons.tile([128, 1], F32)
        nc.vector.memset(epsc, LN_EPS)

        # rhs AP for (t, span) of an interleaved transposed tensor
        def tsl(xt, t, s0, sw):
            return xt[:, s0 // 128:(s0 + sw) // 128, t, :]

        def dma_T(dst, srcn):
            # grouped transpose: src natural [128, NCH, 256] -> dst interleaved
            for g0 in range(0, NCH, 10):
                gn = min(10, NCH - g0)
                nc.sync.dma_start_transpose(
                    out=dst[:, g0:g0 + gn, :, :], in_=srcn[:, g0:g0 + gn, :])

        def dma_Tinv(dstn, src):
            # grouped transpose: src interleaved -> dst natural
            for g0 in range(0, NCH, 10):
                gn = min(10, NCH - g0)
                nc.sync.dma_start_transpose(
                    out=dstn[:, g0:g0 + gn, :, :], in_=src[:, g0:g0 + gn, :, :])

        # ---- load input, cast bf16, one batched transpose -> xT ----
        xnat = big.tile([128, NCH, 256], BF16, tag="big")
        for c in range(0, NCH, 2):
            cn = min(2, NCH - c)
            xin = sm.tile([128, 2, 256], F32, tag="xin")
            for j in range(cn):
                rows = last_rows if c + j == NCH - 1 else 128
                if rows < 128:
                    nc.vector.memset(xin[:, j, :], 0.0)
                nc.sync.dma_start(out=xin[:rows, j, :],
                                  in_=x_in[128 * (c + j):128 * (c + j) + rows, :])
            nc.vector.tensor_copy(out=xnat[:, c:c + cn, :], in_=xin[:, :cn, :])
        xT = xtp.tile([128, NCH, 2, 128], BF16, tag="xT")
        dma_T(xT, xnat)

        # warm up the collective path off the critical path (layer 0 is
        # 'self'; first real AllReduce is layer 1)
        ccw_t = sm.tile([128, 8], F32, tag="ccw")
        nc.vector.memset(ccw_t, 0.0)
        nc.gpsimd.dma_start(out=ccw_in[:, :], in_=ccw_t)
        nc.gpsimd.collective_compute(
            "AllReduce", ALU.add, ins=[ccw_in[:, :]], outs=[ccw_out[:, :]],
            replica_groups=groups)
        ccw_r = sm.tile([128, 8], F32, tag="ccwr")
        nc.gpsimd.dma_start(out=ccw_r, in_=ccw_out[:, :])

        cross_idx = 0
        for li, kind in enumerate(kinds):
            wq = wpool.tile([128, 2, 256], BF16, tag="wq")
            wkv = wpool.tile([128, 2, 512], BF16, tag="wkv")
            wm = wpool.tile([128, 2, 256], BF16, tag="wm")
            w1 = wpool.tile([128, 4, 512], BF16, tag="w1")
            w2 = wpool.tile([128, 4, 256], BF16, tag="w2")
            b1s = wpool.tile([128, 4], F32, tag="b1s")
            for sb_t, dr in ((wq, wq_in), (wkv, wkv_in), (wm, wm_in),
                             (w1, w1_in), (w2, w2_in)):
                nc.sync.dma_start(out=sb_t, in_=dr[li].rearrange("t p n -> p t n"))
            nc.sync.dma_start(out=b1s, in_=b1_in[li].rearrange("(m p) -> p m", p=128))
            if g2b2_general:
                g2r = wpool.tile([128, 256], F32, tag="g2r")
                b2r = wpool.tile([128, 256], F32, tag="b2r")
                nc.sync.dma_start(out=g2r,
                                  in_=g2_in[li:li + 1, :].to_broadcast((128, 256)))
                nc.sync.dma_start(out=b2r,
                                  in_=b2_in[li:li + 1, :].to_broadcast((128, 256)))

            # ---- stage B (first half): q^T + elu -> QT ----
            QT = qp.tile([128, NCH, 2, 128], BF16, tag="QT")

            def emit_B(s0, sw):
                ncl = sw // 128
                qp2 = ps2.tile([128, 2, 512], F32, tag="big2")
                for m in range(2):
                    for t in range(2):
                        nc.tensor.matmul(qp2[:, m, :sw],
                                         wq[:, t, 128 * m:128 * (m + 1)],
                                         tsl(xT, t, s0, sw),
                                         start=(t == 0), stop=(t == 1))
                e2 = sm.tile([128, 2, 512], F32, tag="eQ", bufs=2)
                nc.scalar.activation(out=e2[:, :, :sw], in_=qp2[:, :, :sw],
                                     func=AF.Exp)
                rr = sm.tile([128, 2, 512], F32, tag="rQ", bufs=2)
                nc.scalar.activation(out=rr[:, 0, :sw], in_=qp2[:, 0, :sw],
                                     func=AF.Relu)
                nc.vector.tensor_scalar(out=rr[:, 1, :sw], in0=qp2[:, 1, :sw],
                                        scalar1=0.0, scalar2=None, op0=ALU.max)
                for m in range(2):
                    nc.vector.scalar_tensor_tensor(
                        out=QT[:, s0 // 128:s0 // 128 + ncl, m, :],
                        in0=e2[:, m, :sw], scalar=1.0,
                        in1=rr[:, m, :sw], op0=ALU.min, op1=ALU.add)

            # ---- stage A: k|v fused matmul on 2-chunk pairs; elu(K);
            # stats interleaved.  elu(x)+1 = min(exp(x),1) + max(x,0) ----
            K = big.tile([128, NCH, 256], BF16, tag="big")
            V = big.tile([128, NCH, 2, 129], BF16, tag="big")
            nc.vector.memset(V[:, :, :, 128:129], 1.0)
            st = psst.tile([128, 2, 512], F32, tag="st")

            def emit_stats(c0, pn):
                for j in range(pn):
                    c = c0 + j
                    for h in range(2):
                        nc.tensor.matmul(st[:, h, 0:129],
                                         K[:, c, 128 * h:128 * (h + 1)],
                                         V[:, c, h, :],
                                         start=(c == 0), stop=(c == NCH - 1))

            pairs = [(c0, min(2, NCH - c0)) for c0 in range(0, NCH, 2)]
            for pi, (c0, pn) in enumerate(pairs):
                kvp = ps2.tile([128, 2, 512], F32, tag="big2")
                for j in range(pn):
                    for t in range(2):
                        nc.tensor.matmul(kvp[:, j, :], xT[:, c0 + j, t, :],
                                         wkv[:, t, :],
                                         start=(t == 0), stop=(t == 1))
                kk = kvp[:, :pn, 0:256]
                # elu(k)+1 = min(exp(k),1) + max(k,0); f32 intermediates.
                # exp and relu both read PSUM directly (slot frees after 2
                # parallel hops), one stt combines from SBUF.
                e2 = sm.tile([128, 2, 256], F32, tag="eK", bufs=2)
                nc.scalar.activation(out=e2[:, :pn, :], in_=kk, func=AF.Exp)
                rr = sm.tile([128, 2, 256], F32, tag="rK", bufs=2)
                nc.vector.tensor_scalar(out=rr[:, :pn, :], in0=kk,
                                        scalar1=0.0, scalar2=None, op0=ALU.max)
                nc.scalar.copy(out=V[:, c0:c0 + pn, :, 0:128],
                               in_=kvp[:, :pn, 256:512])
                nc.vector.scalar_tensor_tensor(
                    out=K[:, c0:c0 + pn, :], in0=e2[:, :pn, :], scalar=1.0,
                    in1=rr[:, :pn, :], op0=ALU.min, op1=ALU.add)
                if c0 + pn == NCH and last_rows < 128:
                    nc.vector.memset(K[last_rows:128, NCH - 1, :], 0.0)
                # stats matmuls trail by 2 pairs so the PE never waits on
                # the elu chain
                if pi >= 2:
                    emit_stats(*pairs[pi - 2])
            emit_stats(*pairs[-2])
            emit_stats(*pairs[-1])
            stats = stp.tile([128, 2, 129], F32, tag="stats")
            nc.vector.tensor_tensor(out=stats, in0=st[:, :, 0:129],
                                    in1=maskc, op=ALU.mult)

            if kind == "cross":
                nc.gpsimd.dma_start(out=cc_in[cross_idx][:, :],
                                    in_=stats.rearrange("p a b -> p (a b)"))
                nc.gpsimd.collective_compute(
                    "AllReduce", ALU.add,
                    ins=[cc_in[cross_idx][:, :]],
                    outs=[cc_out[cross_idx][:, :]],
                    replica_groups=groups)

            # ---- stage B: full run after A so the cross-layer AllReduce
            # hides under all 10 spans of elu work ----
            for (s0, sw) in spans:
                emit_B(s0, sw)

            if kind == "cross":
                ssum = stp.tile([128, 2, 129], F32, tag="ssum")
                nc.gpsimd.dma_start(out=ssum.rearrange("p a b -> p (a b)"),
                                    in_=cc_out[cross_idx][:, :])
                pstats = stp.tile([128, 2, 129], F32, tag="pstats")
                nc.vector.tensor_sub(pstats, ssum, stats)
                cross_idx += 1
            else:
                pstats = stats

            kvbd = stp.tile([128, 2, 128], BF16, tag="kvbd")
            ksE = stp.tile([128, 2, 128], BF16, tag="ksE")
            nc.vector.memset(ksE, 0.0)
            for h in range(2):
                nc.vector.tensor_copy(out=kvbd[:, h, :], in_=pstats[:, h, 0:128])
                for b in range(4):
                    bs = slice(32 * b, 32 * (b + 1))
                    src = pstats[bs, h, 128:129]
                    src_b = bass.AP(tensor=src.tensor, offset=src.offset,
                                    ap=[src.ap[0], [0, 32]])
                    nc.vector.tensor_copy(out=ksE[bs, h, bs], in_=src_b)

            # ---- stage C: den + msg matmuls, msgT = msg / den ----
            msgT = big.tile([128, NCH, 2, 128], BF16, tag="big")
            for (s0, sw) in spans:
                for h in range(2):
                    dm = ps2.tile([128, 2, 512], F32, tag="big2")
                    nc.tensor.matmul(dm[:, 0, :sw], ksE[:, h, :],
                                     tsl(QT, h, s0, sw), start=True, stop=True)
                    nc.tensor.matmul(dm[:, 1, :sw], kvbd[:, h, :],
                                     tsl(QT, h, s0, sw), start=True, stop=True)
                    zr = sm.tile([128, 512], F32, tag="den", bufs=2)
                    nc.vector.reciprocal_approx_fast(out=zr[:, :sw],
                                                     in_=dm[:, 0, :sw])
                    nc.vector.tensor_tensor(out=tsl(msgT, h, s0, sw),
                                            in0=dm[:, 1, :sw], in1=zr[:, :sw],
                                            op=ALU.mult)

            # ---- stage D: merge natural + LN1 -> msgLN (4-chunk groups,
            # 2-chunk psum tiles + batched bn_stats / rstd ops) ----
            msgLN = big.tile([128, NCH, 256], BF16, tag="big")
            for g0 in range(0, NCH, 4):
                gn = min(4, NCH - g0)
                st6 = sm.tile([128, 4, 6], F32, tag="st6")
                mvb = sm.tile([128, 4, 2], F32, tag="mvb")
                md = ps2.tile([128, 4, 256], F32, tag="big2")
                for j in range(gn):
                    c = g0 + j
                    for t in range(2):
                        nc.tensor.matmul(md[:, j, :], msgT[:, c, t, :],
                                         wm[:, t, :],
                                         start=(t == 0), stop=(t == 1))
                    nc.vector.bn_stats(out=st6[:, j, :], in_=md[:, j, :])
                for j in range(gn):
                    nc.vector.bn_aggr(out=mvb[:, j, :], in_=st6[:, j, :])
                lnv = sm.tile([128, 4], F32, tag="lnv")
                nc.scalar.activation(out=lnv[:, :gn], in_=mvb[:, :gn, 1],
                                     func=AF.Ln, bias=epsc[:, :])
                rstd = sm.tile([128, 4], F32, tag="rstd")
                nc.scalar.activation(out=rstd[:, :gn], in_=lnv[:, :gn],
                                     func=AF.Exp, scale=-0.5)
                nmr = sm.tile([128, 4], F32, tag="nmr")
                nc.vector.scalar_tensor_tensor(out=nmr[:, :gn],
                                               in0=mvb[:, :gn, 0],
                                               scalar=-1.0, in1=rstd[:, :gn],
                                               op0=ALU.mult, op1=ALU.mult)
                for j in range(gn):
                    c = g0 + j
                    if j % 4 == 3:
                        nc.vector.tensor_scalar(
                            out=msgLN[:, c, :], in0=md[:, j, :],
                            scalar1=rstd[:, j:j + 1],
                            scalar2=nmr[:, j:j + 1],
                            op0=ALU.mult, op1=ALU.add)
                    else:
                        nc.scalar.activation(
                            out=msgLN[:, c, :], in_=md[:, j, :],
                            func=AF.Identity, scale=rstd[:, j:j + 1],
                            bias=nmr[:, j:j + 1])

            # ---- stage E: one batched DMA transpose msgLN -> msgLNT ----
            msgLNT = big.tile([128, NCH, 2, 128], BF16, tag="big")
            dma_T(msgLNT, msgLN)

            # ---- stages F+G, G deferred one span so its matmuls never
            # stall the PE on F's relu chain ----
            h2LN = big.tile([128, NCH, 256], BF16, tag="big")

            def emit_F(s0, sw):
                h1s = h1p.tile([128, 4, 512], BF16, tag="h1s")
                for m in range(4):
                    hps = psm.tile([128, 512], F32, tag="med")
                    for t in range(4):
                        rhs = (tsl(xT, t, s0, sw) if t < 2
                               else tsl(msgLNT, t - 2, s0, sw))
                        nc.tensor.matmul(hps[:, :sw],
                                         w1[:, t, 128 * m:128 * (m + 1)],
                                         rhs, start=(t == 0), stop=(t == 3))
                    if m == 3:
                        nc.vector.tensor_scalar(
                            out=h1s[:, m, :sw], in0=hps[:, :sw],
                            scalar1=b1s[:, m:m + 1], scalar2=0.0,
                            op0=ALU.add, op1=ALU.max)
                    else:
                        nc.scalar.activation(out=h1s[:, m, :sw],
                                             in_=hps[:, :sw], func=AF.Relu,
                                             bias=b1s[:, m:m + 1])
                return h1s

            def emit_G(s0, sw, h1s):
                ncl = sw // 128
                g2t = ps2.tile([128, 4, 256], F32, tag="big2")
                st6 = sm.tile([128, 4, 6], F32, tag="st6")
                mvb = sm.tile([128, 4, 2], F32, tag="mvb")
                for cl in range(ncl):
                    cls = slice(128 * cl, 128 * (cl + 1))
                    for t in range(4):
                        nc.tensor.matmul(g2t[:, cl, :], h1s[:, t, cls],
                                         w2[:, t, :],
                                         start=(t == 0), stop=(t == 3))
                for cl in range(ncl):
                    nc.vector.bn_stats(out=st6[:, cl, :], in_=g2t[:, cl, :])
                    nc.vector.bn_aggr(out=mvb[:, cl, :], in_=st6[:, cl, :])
                lnv = sm.tile([128, 4], F32, tag="lnv")
                nc.scalar.activation(out=lnv[:, :ncl], in_=mvb[:, :ncl, 1],
                                     func=AF.Ln, bias=epsc[:, :])
                rstd = sm.tile([128, 4], F32, tag="rstd")
                nc.scalar.activation(out=rstd[:, :ncl], in_=lnv[:, :ncl],
                                     func=AF.Exp, scale=-0.5)
                nmr = sm.tile([128, 4], F32, tag="nmr")
                nc.vector.scalar_tensor_tensor(out=nmr[:, :ncl],
                                               in0=mvb[:, :ncl, 0],
                                               scalar=-1.0, in1=rstd[:, :ncl],
                                               op0=ALU.mult, op1=ALU.mult)
                for cl in range(ncl):
                    c = s0 // 128 + cl
                    if g2b2_general:
                        hn_t = sm.tile([128, 256], F32, tag="hn")
                        nc.scalar.activation(out=hn_t, in_=g2t[:, cl, :],
                                             func=AF.Identity,
                                             scale=rstd[:, cl:cl + 1],
                                             bias=nmr[:, cl:cl + 1])
                        hg = sm.tile([128, 256], F32, tag="hg")
                        nc.vector.tensor_tensor(out=hg, in0=hn_t, in1=g2r,
                                                op=ALU.mult)
                        nc.vector.tensor_tensor(out=h2LN[:, c, :], in0=hg,
                                                in1=b2r, op=ALU.add)
                    else:
                        nc.scalar.activation(out=h2LN[:, c, :], in_=g2t[:, cl, :],
                                             func=AF.Identity,
                                             scale=rstd[:, cl:cl + 1],
                                             bias=nmr[:, cl:cl + 1])

            # H groups (transpose + residual add) interleave into the F/G
            # span loop; the small 2-chunk tail keeps the layer boundary off
            # the PE critical path.
            h2TT = big.tile([128, NCH, 2, 128], BF16, tag="big")
            xT_new = xtp.tile([128, NCH, 2, 128], BF16, tag="xT")
            h_groups = [(0, 10), (10, 10), (20, 10), (30, 6), (36, 2)]
            h_next = 0

            def emit_H(gi):
                g0, gn = h_groups[gi]
                gs = slice(g0, g0 + gn)
                nc.sync.dma_start_transpose(
                    out=h2TT[:, gs, :, :], in_=h2LN[:, gs, :])
                eng = nc.vector
                eng.tensor_tensor(out=xT_new[:, gs, :, :],
                                  in0=h2TT[:, gs, :, :],
                                  in1=xT[:, gs, :, :], op=ALU.add)

            prevF = None
            for (s0, sw) in spans:
                h1s = emit_F(s0, sw)
                if prevF is not None:
                    emit_G(*prevF)
                    done = prevF[0] // 128 + prevF[1] // 128
                    while (h_next < len(h_groups)
                           and h_groups[h_next][0] + h_groups[h_next][1] <= done):
                        emit_H(h_next)
                        h_next += 1
                prevF = (s0, sw, h1s)
            emit_G(*prevF)
            while h_next < len(h_groups):
                emit_H(h_next)
                h_next += 1
            xT = xT_new

        # ---- output: one DMA transpose back + cast f32 + DMA out ----
        onat = big.tile([128, NCH, 2, 128], BF16, tag="big")
        dma_Tinv(onat, xT)
        for c in range(0, NCH, 2):
            cn = min(2, NCH - c)
            onf = sm.tile([128, 2, 256], F32, tag="xin")
            nc.vector.tensor_copy(out=onf[:, :cn, :], in_=onat[:, c:c + cn, :, :])
            for j in range(cn):
                rows = last_rows if c + j == NCH - 1 else 128
                nc.sync.dma_start(out=out_d[128 * (c + j):128 * (c + j) + rows, :],
                                  in_=onf[:rows, j, :])

    nc.compile()
    return nc


L_SEQ = 4800
LAYER_KINDS = ("self", "cross", "self", "cross", "self", "cross", "self", "cross")

_cache = {}


def _get_program(weights):
    key = id(weights[0])
    if key not in _cache:
        _cache.clear()
        wmap, g2b2_general = prep_weights(*weights)
        prog = build(L_SEQ, LAYER_KINDS, g2b2_general)
        _cache[key] = (prog, wmap)
    return _cache[key]


def kernel(feat0, feat1, Wq, Wk, Wv, Wm, W1, W2, g1, b1, g2, b2):
    feat0 = np.asarray(feat0, dtype=np.float32)
    feat1 = np.asarray(feat1, dtype=np.float32)
    weights = tuple(np.asarray(w, dtype=np.float32)
                    for w in (Wq, Wk, Wv, Wm, W1, W2, g1, b1, g2, b2))
    prog, wmap = _get_program(weights)

    seqs = np.empty((N_CORES, L_SEQ, D_MODEL), np.float32)
    seqs[0::2] = feat0
    seqs[1::2] = feat1
    in_maps = [dict(x=np.ascontiguousarray(seqs[i]), **wmap)
               for i in range(N_CORES)]

    from concourse.bass_utils import run_bass_kernel_spmd
    res = run_bass_kernel_spmd(prog, in_maps, list(range(N_CORES)))
    out = np.stack([res.results[i]["out"] for i in range(N_CORES)])
    return out[0::2].copy(), out[1::2].copy()
